# revision 1
# baseline (speedup 1.0000x reference)
"""CrossAttentionBlock kernel for 8 Trainium2 NeuronCores.

Full inputs in, full output out. Sharding: data-parallel over the H axis
(each row's WxW attention is independent) -- H=256 rows split 32-per-core
across 8 cores; the small 1x1-conv weights are replicated. Per-core math
runs as one fused XLA-Neuron program; host gathers the 8 output shards.
"""

import numpy as np
import jax
import jax.numpy as jnp
from functools import partial

B, C, H, W = 4, 64, 256, 256
N_CORES = 8
H_SHARD = H // N_CORES  # 32 rows per core
EPS = 1e-6


def _ln2d(x, w, b):
    # LayerNorm over channel axis per spatial location (x: [B,C,h,W])
    mu = jnp.mean(x, axis=1, keepdims=True)
    var = jnp.mean((x - mu) ** 2, axis=1, keepdims=True)
    xn = (x - mu) * jax.lax.rsqrt(var + EPS)
    return xn * w[None, :, None, None] + b[None, :, None, None]


def _conv1x1(x, Wk, b):
    return jnp.einsum('bchw,oc->bohw', x, Wk) + b[None, :, None, None]


@partial(jax.jit, static_argnums=())
def _block(x, wnA, bnA, wnB, bnB, W1A, b1A, W1B, b1B,
           W2A, b2A, W2B, b2B, beta, gamma):
    # x: [B, 2C, h_shard, W] slice; weights replicated
    c = C
    xA, xB = x[:, :c], x[:, c:]
    scale = c ** (-0.5)
    qA = _conv1x1(_ln2d(xA, wnA, bnA), W1A, b1A).transpose(0, 2, 3, 1)  # [B,h,W,C]
    qB = _conv1x1(_ln2d(xB, wnB, bnB), W1B, b1B).transpose(0, 2, 1, 3)  # [B,h,C,W]
    vA = _conv1x1(xA, W2A, b2A).transpose(0, 2, 3, 1)                   # [B,h,W,C]
    vB = _conv1x1(xB, W2B, b2B).transpose(0, 2, 3, 1)                   # [B,h,W,C]
    att = jnp.einsum('bhwc,bhcx->bhwx', qA, qB) * scale                 # [B,h,W,W]
    fA = jnp.einsum('bhwx,bhxc->bhwc', jax.nn.softmax(att, axis=-1), vB)
    attT = jnp.swapaxes(att, -1, -2)
    fB = jnp.einsum('bhwx,bhxc->bhwc', jax.nn.softmax(attT, axis=-1), vA)
    fA = fA.transpose(0, 3, 1, 2) * beta   # [B,C,h,W]
    fB = fB.transpose(0, 3, 1, 2) * gamma
    return jnp.concatenate((xA + fA, xB + fB), axis=1)


def kernel(x, wnA, bnA, wnB, bnB, W1A, b1A, W1B, b1B,
           W2A, b2A, W2B, b2B, beta, gamma):
    x = np.asarray(x, dtype=np.float32)
    weights = [np.asarray(a, dtype=np.float32) for a in
               (wnA, bnA, wnB, bnB, W1A, b1A, W1B, b1B,
                W2A, b2A, W2B, b2B, beta, gamma)]

    devs = jax.devices()[:N_CORES]

    # Ship shards + replicated weights to each core, dispatch async.
    outs = []
    for k, d in enumerate(devs):
        xs = jax.device_put(x[:, :, k * H_SHARD:(k + 1) * H_SHARD, :], d)
        ws = [jax.device_put(w, d) for w in weights]
        outs.append(_block(xs, *ws))

    # Gather: block on all shards, concat along H.
    outs = [np.asarray(o) for o in outs]
    return np.concatenate(outs, axis=2).astype(np.float32)


# revision 2
# speedup vs baseline: 1.2696x; 1.2696x over previous
"""CrossAttentionBlock kernel for 8 Trainium2 NeuronCores.

Full inputs in, full output out. Sharding: data-parallel over the H axis
(each row's WxW attention is independent) -- H=256 rows split 32-per-core
across 8 cores; the small 1x1-conv weights are replicated. Per-core math
runs as one fused XLA-Neuron program; host gathers the 8 output shards.
"""

import numpy as np
import jax
import jax.numpy as jnp
from functools import partial

try:  # reuse compiled executables across processes/directories
    jax.config.update("jax_compilation_cache_dir", "/tmp/jax_comp_cache")
except Exception:
    pass

B, C, H, W = 4, 64, 256, 256
N_CORES = 8
H_SHARD = H // N_CORES  # 32 rows per core
EPS = 1e-6


def _ln2d(x, w, b):
    # LayerNorm over channel axis per spatial location (x: [B,C,h,W])
    mu = jnp.mean(x, axis=1, keepdims=True)
    var = jnp.mean((x - mu) ** 2, axis=1, keepdims=True)
    xn = (x - mu) * jax.lax.rsqrt(var + EPS)
    return xn * w[None, :, None, None] + b[None, :, None, None]


def _conv1x1(x, Wk, b):
    return jnp.einsum('bchw,oc->bohw', x, Wk) + b[None, :, None, None]


@partial(jax.jit, static_argnums=())
def _block(x, wnA, bnA, wnB, bnB, W1A, b1A, W1B, b1B,
           W2A, b2A, W2B, b2B, beta, gamma):
    # x: [B, 2C, h_shard, W] slice; weights replicated
    c = C
    xA, xB = x[:, :c], x[:, c:]
    scale = c ** (-0.5)
    qA = _conv1x1(_ln2d(xA, wnA, bnA), W1A, b1A).transpose(0, 2, 3, 1)  # [B,h,W,C]
    qB = _conv1x1(_ln2d(xB, wnB, bnB), W1B, b1B).transpose(0, 2, 1, 3)  # [B,h,C,W]
    vA = _conv1x1(xA, W2A, b2A).transpose(0, 2, 3, 1)                   # [B,h,W,C]
    vB = _conv1x1(xB, W2B, b2B).transpose(0, 2, 3, 1)                   # [B,h,W,C]
    att = jnp.einsum('bhwc,bhcx->bhwx', qA, qB) * scale                 # [B,h,W,W]
    fA = jnp.einsum('bhwx,bhxc->bhwc', jax.nn.softmax(att, axis=-1), vB)
    attT = jnp.swapaxes(att, -1, -2)
    fB = jnp.einsum('bhwx,bhxc->bhwc', jax.nn.softmax(attT, axis=-1), vA)
    fA = fA.transpose(0, 3, 1, 2) * beta   # [B,C,h,W]
    fB = fB.transpose(0, 3, 1, 2) * gamma
    return jnp.concatenate((xA + fA, xB + fB), axis=1)


def kernel(x, wnA, bnA, wnB, bnB, W1A, b1A, W1B, b1B,
           W2A, b2A, W2B, b2B, beta, gamma):
    x = np.asarray(x, dtype=np.float32)
    weights = [np.asarray(a, dtype=np.float32) for a in
               (wnA, bnA, wnB, bnB, W1A, b1A, W1B, b1B,
                W2A, b2A, W2B, b2B, beta, gamma)]

    devs = jax.devices()[:N_CORES]

    # Ship shards + replicated weights to each core, dispatch async.
    outs = []
    for k, d in enumerate(devs):
        xs = jax.device_put(x[:, :, k * H_SHARD:(k + 1) * H_SHARD, :], d)
        ws = [jax.device_put(w, d) for w in weights]
        outs.append(_block(xs, *ws))

    # Gather: block on all shards, concat along H.
    outs = [np.asarray(o) for o in outs]
    return np.concatenate(outs, axis=2).astype(np.float32)


# revision 3
# speedup vs baseline: 34.1295x; 26.8825x over previous
"""CrossAttentionBlock kernel for 8 Trainium2 NeuronCores.

Full inputs in, full output out. Sharding: data-parallel over the H axis
(each row's WxW attention is independent) -- H=256 rows split 32-per-core
across 8 cores; the small 1x1-conv weights are replicated (shipped as one
packed array per core, unpacked on device). Per-core math runs as one
fused XLA-Neuron program; transfers, dispatches and gathers for the 8
cores run concurrently; host concatenates the 8 output shards.
"""

from concurrent.futures import ThreadPoolExecutor
from functools import partial

import numpy as np
import jax
import jax.numpy as jnp

try:  # reuse compiled executables across processes/directories
    jax.config.update("jax_compilation_cache_dir", "/tmp/jax_comp_cache")
except Exception:
    pass

B, C, H, W = 4, 64, 256, 256
N_CORES = 8
H_SHARD = H // N_CORES  # 32 rows per core
EPS = 1e-6

# (name, shape) of the replicated weights, in kernel-arg order
_WSPECS = [
    ('wnA', (C,)), ('bnA', (C,)), ('wnB', (C,)), ('bnB', (C,)),
    ('W1A', (C, C)), ('b1A', (C,)), ('W1B', (C, C)), ('b1B', (C,)),
    ('W2A', (C, C)), ('b2A', (C,)), ('W2B', (C, C)), ('b2B', (C,)),
    ('beta', (1, C, 1, 1)), ('gamma', (1, C, 1, 1)),
]


def _ln2d(x, w, b):
    # LayerNorm over channel axis per spatial location (x: [B,C,h,W])
    mu = jnp.mean(x, axis=1, keepdims=True)
    var = jnp.mean((x - mu) ** 2, axis=1, keepdims=True)
    xn = (x - mu) * jax.lax.rsqrt(var + EPS)
    return xn * w[None, :, None, None] + b[None, :, None, None]


def _conv1x1(x, Wk, b):
    return jnp.einsum('bchw,oc->bohw', x, Wk) + b[None, :, None, None]


@jax.jit
def _unpack(wflat):
    out = []
    off = 0
    for _, shp in _WSPECS:
        n = int(np.prod(shp))
        out.append(wflat[off:off + n].reshape(shp))
        off += n
    return tuple(out)


@partial(jax.jit, static_argnums=())
def _block(x, wnA, bnA, wnB, bnB, W1A, b1A, W1B, b1B,
           W2A, b2A, W2B, b2B, beta, gamma):
    # x: [B, 2C, h_shard, W] slice; weights replicated
    c = C
    xA, xB = x[:, :c], x[:, c:]
    scale = c ** (-0.5)
    qA = _conv1x1(_ln2d(xA, wnA, bnA), W1A, b1A).transpose(0, 2, 3, 1)  # [B,h,W,C]
    qB = _conv1x1(_ln2d(xB, wnB, bnB), W1B, b1B).transpose(0, 2, 1, 3)  # [B,h,C,W]
    vA = _conv1x1(xA, W2A, b2A).transpose(0, 2, 3, 1)                   # [B,h,W,C]
    vB = _conv1x1(xB, W2B, b2B).transpose(0, 2, 3, 1)                   # [B,h,W,C]
    att = jnp.einsum('bhwc,bhcx->bhwx', qA, qB) * scale                 # [B,h,W,W]
    fA = jnp.einsum('bhwx,bhxc->bhwc', jax.nn.softmax(att, axis=-1), vB)
    attT = jnp.swapaxes(att, -1, -2)
    fB = jnp.einsum('bhwx,bhxc->bhwc', jax.nn.softmax(attT, axis=-1), vA)
    fA = fA.transpose(0, 3, 1, 2) * beta   # [B,C,h,W]
    fB = fB.transpose(0, 3, 1, 2) * gamma
    return jnp.concatenate((xA + fA, xB + fB), axis=1)


def kernel(x, wnA, bnA, wnB, bnB, W1A, b1A, W1B, b1B,
           W2A, b2A, W2B, b2B, beta, gamma):
    x = np.ascontiguousarray(np.asarray(x, dtype=np.float32))
    wmap = dict(wnA=wnA, bnA=bnA, wnB=wnB, bnB=bnB, W1A=W1A, b1A=b1A,
                W1B=W1B, b1B=b1B, W2A=W2A, b2A=b2A, W2B=W2B, b2B=b2B,
                beta=beta, gamma=gamma)
    wflat = np.concatenate(
        [np.asarray(wmap[n], np.float32).reshape(-1) for n, _ in _WSPECS])

    devs = jax.devices()[:N_CORES]

    def run_core(k):
        d = devs[k]
        xs = jax.device_put(x[:, :, k * H_SHARD:(k + 1) * H_SHARD, :], d)
        wf = jax.device_put(wflat, d)
        ws = _unpack(wf)
        return _block(xs, *ws)

    with ThreadPoolExecutor(N_CORES) as ex:
        outs = list(ex.map(run_core, range(N_CORES)))
        shards = list(ex.map(np.asarray, outs))

    return np.concatenate(shards, axis=2).astype(np.float32)
